# revision 7
# baseline (speedup 1.0000x reference)
"""Multi-head causal self-attention with RoPE on 8 Trainium2 NeuronCores.

Problem: x[2,2048,2048], wq/wk/wv/wo[2048,2048] fp32, 16 heads (hd=128),
interleaved RoPE, causal softmax, Megatron-style tensor parallelism over
heads: 2 heads per core, wo row-sharded, partial outputs summed on host.

All matmul operands are bf16 (measured end-to-end rel err ~3e-3 vs the
2e-2 gate); PSUM accumulation is fp32. bf16 stationaries get the
compiler's fast-weight-load path so LDWEIGHTS hides under the moving
stream, and all DMA volumes halve vs fp32.

Layout strategy (per core, per batch b):
  - host packs x chunk-major in exact SBUF image (xck[c] = [p, dt, s])
    so every chunk load is one DMA with 16KB-contiguous per-partition
    runs (max packet size); wqkv/wo likewise packed to SBUF image.
  - cold start: chunk-0 projections run dt-group-major across all 4
    qk psum chains + all 4 v chains simultaneously (8 PSUM banks), so
    the PE consumes x/w dt-groups as the interleaved startup DMAs land.
  - projections: qT,kT per head via lhsT=w-tile [d,e], rhs=x [d,s512]
    -> q^T,k^T [e=128, s] directly; v natural [s, e] via lhsT=x-subtile;
    RoPE fused right after each projection chunk (rot matmul + DVE,
    cos/sin tables in bf16 for 2x DVE mode).
  - attention per (b, j-block of 512 q), heads interleaved and scores
    pipelined three kv-tiles ahead so the exp (ACT) latency never stalls
    the PE:
      scoresT[kv=128, q<=512] = kT-tile.T @ qT-block
      attn = exp(scoresT) on ACT (bf16); triangle mask on 128-col band
      oT[d, q] += v-tile.T @ attn
      rowsum: full kv-tiles pair+quad-summed on DVE, one ones-matmul
      per quad; diagonal tiles matmul'd individually
      oT_norm = oT * reciprocal_approx_fast(rowsum) (DVE, bf16)
  - output projection per j-block: yT[e, jsl] = sum_ct woT-tile.T @ oT,
    copied PSUM->SBUF on DVE into [P,2,SC] tiles, one DMA per et-pair
  - host: y = sum over cores of yT^T
"""

import os
import sys

for _p in ("/opt/trn_rl_repo", "/root/.axon_site/_ro/trn_rl_repo"):
    if os.path.isdir(_p) and _p not in sys.path:
        sys.path.append(_p)

import numpy as np

import concourse.bacc as bacc
import concourse.mybir as mybir
import concourse.tile as tile
from concourse.alu_op_type import AluOpType
from concourse.bass_utils import run_bass_kernel_spmd

F32 = mybir.dt.float32
BF16 = mybir.dt.bfloat16

B, S, D = 2, 2048, 2048
H, HD = 16, 128
NCORES = 8
HPC = H // NCORES            # heads per core = 2
CPC = HPC * HD               # channels per core = 256
P = 128
SC = 512                     # s-chunk for projections / q-block for attention
NSC = S // SC                # 4
NDT = D // P                 # 16 contraction tiles
ROPE_THETA = 10000.0

# cold-start dt-groups: small first so the PE starts as soon as the first
# x/w tiles land, then steady 4-tile groups paced to the DMA delivery
XGRP = [(0, 2), (2, 2), (4, 4), (8, 4), (12, 4)]

Exp = mybir.ActivationFunctionType.Exp

last_exec_time_ns = None
_nc_cache = None


def _build_nc():
    nc = bacc.Bacc("TRN2", target_bir_lowering=False, debug=False)

    # host-packed SBUF-image inputs (contiguous per-partition runs)
    xck = nc.dram_tensor("xck", [B * NSC, P, NDT, SC], BF16, kind="ExternalInput")
    wqkvp = nc.dram_tensor("wqkvp", [P, NDT, 6 * P], BF16, kind="ExternalInput")
    wop = nc.dram_tensor("wop", [P, CPC // P, D], BF16, kind="ExternalInput")
    cosT = nc.dram_tensor("cosT", [HD, S], BF16, kind="ExternalInput")
    sinT = nc.dram_tensor("sinT", [HD, S], BF16, kind="ExternalInput")
    rotL = nc.dram_tensor("rotL", [HD, HD], BF16, kind="ExternalInput")
    trimask = nc.dram_tensor("trimask", [P, P], BF16, kind="ExternalInput")
    ones = nc.dram_tensor("ones", [P, P], BF16, kind="ExternalInput")
    yTp = nc.dram_tensor("yTp", [B, NSC, NDT // 4, P, 4, SC], BF16,
                         kind="ExternalOutput")

    with tile.TileContext(nc) as tc:
        with tc.tile_pool(name="const", bufs=1) as constp, \
             tc.tile_pool(name="xp", bufs=3) as xp, \
             tc.tile_pool(name="qk", bufs=2) as qkp, \
             tc.tile_pool(name="vp", bufs=2) as vp, \
             tc.tile_pool(name="op", bufs=2) as op_, \
             tc.tile_pool(name="attn", bufs=13) as attnp, \
             tc.tile_pool(name="asum", bufs=6) as sump, \
             tc.tile_pool(name="tmp", bufs=4) as tmpp, \
             tc.tile_pool(name="yt", bufs=5) as ytp, \
             tc.tile_pool(name="ps", bufs=4, space="PSUM") as psp, \
             tc.tile_pool(name="acc", bufs=4, space="PSUM") as accp:

            wq_sb = constp.tile([P, NDT, 6 * P], BF16)
            wo_sb = constp.tile([P, CPC // P, D], BF16)
            cos_sb = constp.tile([P, S], BF16)
            sin_sb = constp.tile([P, S], BF16)
            rot_sb = constp.tile([P, P], BF16)
            mask_sb = constp.tile([P, P], BF16)
            ones_sb = constp.tile([P, P], BF16)

            chunk_tiles = {}

            def chunk_dma(c, queue):
                xt = xp.tile([P, NDT, SC], BF16, tag="xt", name=f"xt{c}")
                queue.dma_start(xt[:], xck[c])
                chunk_tiles[c] = xt

            # ---- cold-start DMA schedule: interleave chunk-0 x and wqkv
            #      dt-groups round-robin over 4 queues so the group-major
            #      projection below is fed just-in-time; everything else
            #      (rot, chunk1, cos/sin, wo) queues behind in need order ----
            xt0 = xp.tile([P, NDT, SC], BF16, tag="xt", name="xt0")
            # w before x per group (PE starts with LDWEIGHTS); the scalar
            # queue joins late because its preamble (ACT_TABLE_LOAD) is
            # the longest
            GQ = [(nc.sync, nc.gpsimd), (nc.gpsimd, nc.sync),
                  (nc.scalar, nc.sync), (nc.gpsimd, nc.scalar),
                  (nc.sync, nc.gpsimd)]
            for (gs, gl), (wq_q, x_q) in zip(XGRP, GQ):
                wq_q.dma_start(wq_sb[:, gs:gs + gl, :], wqkvp[:, gs:gs + gl, :])
                x_q.dma_start(xt0[:, gs:gs + gl, :], xck[0, :, gs:gs + gl, :])
            chunk_tiles[0] = xt0
            qi = 0
            nc.sync.dma_start(rot_sb[:], rotL[:])
            nc.gpsimd.dma_start(mask_sb[:], trimask[:])
            nc.scalar.dma_start(ones_sb[:], ones[:])
            # chunk 1 split in 4 so its head tiles land before chunk-0
            # projections finish
            xt1 = xp.tile([P, NDT, SC], BF16, tag="xt", name="xt1")
            Q3 = [nc.sync, nc.gpsimd, nc.scalar]
            for g in range(4):
                Q3[g % 3].dma_start(xt1[:, 4 * g:4 * g + 4, :],
                                    xck[1, :, 4 * g:4 * g + 4, :])
            chunk_tiles[1] = xt1
            nc.scalar.dma_start(cos_sb[:], cosT[:])
            nc.sync.dma_start(sin_sb[:], sinT[:])
            nc.gpsimd.dma_start(wo_sb[:], wop[:])

            qkT = {}
            v_sb = {}

            def emit_rope(b, e, sl, pool):
                pr = pool.tile([P, SC], F32, tag="acc" if pool is accp else "ps")
                nc.tensor.matmul(pr[:], rot_sb[:], qkT[b, e][:, sl],
                                 start=True, stop=True)
                tmp = tmpp.tile([P, SC], BF16, tag="ropetmp")
                nc.vector.tensor_tensor(tmp[:], pr[:], sin_sb[:, sl],
                                        AluOpType.mult)
                nc.vector.tensor_tensor(qkT[b, e][:, sl], qkT[b, e][:, sl],
                                        cos_sb[:, sl], AluOpType.mult)
                nc.vector.tensor_tensor(qkT[b, e][:, sl], qkT[b, e][:, sl],
                                        tmp[:], AluOpType.add)

            def emit_proj_chunk0():
                """Cold-start chunk: dt-group-major across all 8 psum chains
                so the PE consumes x/w tiles in DMA arrival order."""
                xts = chunk_tiles.pop(0)
                sl = slice(0, SC)
                pqs = [psp.tile([P, SC], F32, tag="ps", name=f"pq0_{e}")
                       for e in range(4)]
                pvs = [accp.tile([P, SC], F32, tag="acc", name=f"pv0_{s}")
                       for s in range(4)]
                for (gs, gl) in XGRP:
                    for k in range(gs, gs + gl):
                        for e in range(4):
                            nc.tensor.matmul(pqs[e][:],
                                             wq_sb[:, k, e * P:(e + 1) * P],
                                             xts[:, k, :],
                                             start=(k == 0), stop=(k == NDT - 1),
                                             skip_group_check=True)
                        for ss in range(4):
                            nc.tensor.matmul(pvs[ss][:, :CPC],
                                             xts[:, k, ss * P:(ss + 1) * P],
                                             wq_sb[:, k, 4 * P:6 * P],
                                             start=(k == 0), stop=(k == NDT - 1),
                                             skip_group_check=True)
                for e in range(4):
                    nc.scalar.copy(qkT[0, e][:, sl], pqs[e][:])
                for ss in range(4):
                    nc.scalar.copy(v_sb[0][:, ss, :], pvs[ss][:, :CPC])
                for e in range(4):
                    emit_rope(0, e, sl, accp)
                chunk_dma(2, nc.sync)

            def emit_proj_chunk(b, sc, flush_pending):
                c = b * NSC + sc
                xts = chunk_tiles.pop(c)
                sl = slice(sc * SC, (sc + 1) * SC)
                for e in range(4):
                    pq = psp.tile([P, SC], F32, tag="ps")
                    for dt in range(NDT):
                        nc.tensor.matmul(pq[:],
                                         wq_sb[:, dt, e * P:(e + 1) * P],
                                         xts[:, dt, :],
                                         start=(dt == 0), stop=(dt == NDT - 1))
                    nc.scalar.copy(qkT[b, e][:, sl], pq[:])
                    if e == 0 and flush_pending is not None:
                        flush_pending()
                    if e > 0:
                        emit_rope(b, e - 1, sl, psp)
                for ss in range(SC // P):
                    pv = psp.tile([P, SC], F32, tag="ps")
                    pvv = pv[:, :CPC]
                    for dt in range(NDT):
                        nc.tensor.matmul(pvv,
                                         xts[:, dt, ss * P:(ss + 1) * P],
                                         wq_sb[:, dt, 4 * P:6 * P],
                                         start=(dt == 0), stop=(dt == NDT - 1))
                    if ss == 0:
                        emit_rope(b, 3, sl, psp)
                    nc.scalar.copy(v_sb[b][:, sc * (SC // P) + ss, :], pvv)
                # prefetch chunk c+2 only after chunk c's readers are emitted,
                # so the buffer-reuse WAR dependency is tracked
                if c + 2 < B * NSC:
                    chunk_dma(c + 2, nc.gpsimd if c % 2 else nc.sync)

            pending_oproj = [None]

            def flush_oproj():
                fn = pending_oproj[0]
                if fn is not None:
                    pending_oproj[0] = None
                    fn()

            oT = {}
            for b in range(B):
                for e in range(4):
                    qkT[b, e] = qkp.tile([P, S], BF16, tag=f"qk{e}",
                                         name=f"qkT{b}_{e}")
                v_sb[b] = vp.tile([P, NDT, CPC], BF16, tag="v", name=f"v{b}")
                for sc in range(NSC):
                    if b == 0 and sc == 0:
                        emit_proj_chunk0()
                    else:
                        emit_proj_chunk(b, sc,
                                        flush_oproj if sc == 0 else None)

                # ---- attention: j outer, heads interleaved, scores three
                #      kv-tiles ahead so exp latency is hidden; each j's
                #      output projection is deferred into the next j-block's
                #      (or batch's) pipeline so the PE never waits on the
                #      DVE softmax-normalization chain ----
                oT[b] = [op_.tile([P, S], BF16, tag=f"o{h}", name=f"oT{b}_{h}")
                         for h in range(HPC)]

                def emit_score(j, t, h, b=b):
                    dp = t - (SC // P) * j
                    dlt = max(dp, 0) * P
                    qsl = slice(j * SC + dlt, (j + 1) * SC)
                    pscore = psp.tile([P, SC], F32, tag="ps")
                    nc.tensor.matmul(pscore[:, dlt:],
                                     qkT[b, 2 + h][:, t * P:(t + 1) * P],
                                     qkT[b, h][:, qsl],
                                     start=True, stop=True)
                    at = attnp.tile([P, SC], BF16, tag="attn")
                    nc.scalar.activation(at[:, dlt:], pscore[:, dlt:],
                                         Exp, bias=0.0, scale=1.0)
                    if dp >= 0:  # triangle mask on the 128-col band
                        # gpsimd: its queue is idle in attention windows, so
                        # the masked tile is ready for AV without waiting
                        # behind DVE's cast backlog
                        nc.gpsimd.tensor_tensor(
                            at[:, dlt:dlt + P], at[:, dlt:dlt + P],
                            mask_sb[:], AluOpType.mult)
                    return at

                def make_oproj(j, b=b, in_proj=False, final=False):
                    jsl = slice(j * SC, (j + 1) * SC)

                    def emit():
                        # PSUM->SBUF casts 3:1 DVE:ACT -- ACT's exp-idle
                        # window during a flush only absorbs ~4 casts;
                        # stores are one contiguous [P,4,SC] DMA per 4 ets
                        store_q = ([nc.sync, nc.gpsimd, nc.scalar]
                                   if final else [nc.sync, nc.gpsimd])
                        for eg in range(NDT // 4):
                            yt = ytp.tile([P, 4, SC], BF16, tag="yt")
                            for sub in range(4):
                                et = 4 * eg + sub
                                py = psp.tile([P, SC], F32, tag="ps")
                                for ct in range(HPC):
                                    nc.tensor.matmul(
                                        py[:],
                                        wo_sb[:, ct, et * P:(et + 1) * P],
                                        oT[b][ct][:, jsl],
                                        start=(ct == 0), stop=(ct == HPC - 1))
                                if et % 4 == 3:
                                    nc.scalar.copy(yt[:, sub, :], py[:])
                                else:
                                    nc.vector.tensor_copy(yt[:, sub, :], py[:])
                            store_q[eg % len(store_q)].dma_start(
                                yTp[b, j, eg], yt[:])
                    return emit

                LA = 4  # score lookahead in kv-tiles
                att_q = {}  # (t, h) -> attn tile, pipelined
                at_prev = {}  # stashed full attn tiles awaiting pair-sum
                pair_prev = {}  # stashed pair-sums awaiting quad-sum
                for tp in range(LA):
                    for h in range(HPC):
                        att_q[0, tp, h] = emit_score(0, tp, h)

                for j in range(NSC):
                    jsl = slice(j * SC, (j + 1) * SC)
                    n_kv = (SC // P) * (j + 1)
                    po = [accp.tile([P, SC], F32, tag="acc", name=f"po{h}")
                          for h in range(HPC)]
                    prs = [accp.tile([P, SC], F32, tag="acc", name=f"prs{h}")
                           for h in range(HPC)]
                    for t in range(n_kv):
                        nxt = t + LA
                        if nxt < n_kv:
                            for h in range(HPC):
                                att_q[j, nxt, h] = emit_score(j, nxt, h)
                        elif j + 1 < NSC:
                            for h in range(HPC):
                                att_q[j + 1, nxt - n_kv, h] = emit_score(
                                    j + 1, nxt - n_kv, h)
                        dlt = max(t - (SC // P) * j, 0) * P
                        full = t < (SC // P) * j
                        for h in range(HPC):
                            at = att_q.pop((j, t, h))
                            nc.tensor.matmul(po[h][:, dlt:],
                                             v_sb[b][:, t, h * HD:(h + 1) * HD],
                                             at[:, dlt:],
                                             start=(t == 0), stop=(t == n_kv - 1),
                                             skip_group_check=True)
                            if full and t % 2 == 0:
                                at_prev[h] = at
                            elif full and t % 4 == 1:
                                # first pair of a quad: sum on gpsimd (its
                                # queue has no cast backlog), stash
                                s2 = sump.tile([P, SC], BF16, tag="asum",
                                               name=f"asum{h}")
                                nc.gpsimd.tensor_tensor(
                                    s2[:], at_prev.pop(h)[:], at[:],
                                    AluOpType.add)
                                pair_prev[h] = s2
                            elif full:
                                # second pair: fold into the stashed pair-sum,
                                # then one rowsum matmul streams 1/4 of the
                                # full-tile elements
                                s2 = sump.tile([P, SC], BF16, tag="asum",
                                               name=f"asum{h}")
                                nc.gpsimd.tensor_tensor(
                                    s2[:], at_prev.pop(h)[:], at[:],
                                    AluOpType.add)
                                q4 = pair_prev.pop(h)
                                nc.gpsimd.tensor_tensor(
                                    q4[:], q4[:], s2[:], AluOpType.add)
                                nc.tensor.matmul(prs[h][:], ones_sb[:], q4[:],
                                                 start=(t == 3), stop=False,
                                                 skip_group_check=True)
                            else:
                                nc.tensor.matmul(prs[h][:, dlt:], ones_sb[:],
                                                 at[:, dlt:],
                                                 start=(t == 0),
                                                 stop=(t == n_kv - 1),
                                                 skip_group_check=True)
                        if t == 1:
                            flush_oproj()
                    for h in range(HPC):
                        recip = tmpp.tile([P, SC], F32, tag="recip")
                        nc.vector.reciprocal_approx_fast(recip[:], prs[h][:])
                        nc.vector.tensor_tensor(oT[b][h][:, jsl], po[h][:],
                                                recip[:], AluOpType.mult)
                    pending_oproj[0] = make_oproj(
                        j, in_proj=(j == NSC - 1),
                        final=(b == B - 1 and j == NSC - 1))
            flush_oproj()
    nc.finalize()
    return nc


def _host_inputs(x, wq, wk, wv, wo):
    """Build per-core input maps (host-side shard + SBUF-image packing)."""
    import ml_dtypes
    bf16 = ml_dtypes.bfloat16
    scale = 1.0 / np.sqrt(np.float32(HD))

    # x packed chunk-major in SBUF image: xck[b*NSC+sc] = [p, dt, s]
    xr = x.reshape(B, NSC, SC, NDT, P)            # (b, sc, s, o, p)
    xck = np.ascontiguousarray(
        xr.transpose(0, 1, 4, 3, 2).reshape(B * NSC, P, NDT, SC)).astype(bf16)

    # RoPE tables in [e, s] layout (same for every head), bf16 for 2x DVE
    inv_freq = 1.0 / (ROPE_THETA ** (np.arange(0, HD, 2, dtype=np.float64) / HD))
    ang = np.arange(S, dtype=np.float64)[None, :] * inv_freq[:, None]  # [64, S]
    cosT = np.repeat(np.cos(ang), 2, axis=0).astype(bf16)  # [128, S]
    sinT = np.repeat(np.sin(ang), 2, axis=0).astype(bf16)

    # signed pair-swap: qrot[2i] = -q[2i+1], qrot[2i+1] = q[2i]
    # matmul computes qrot[m, s] = sum_k rotL[k, m] q[k, s]
    rotL = np.zeros((HD, HD), dtype=np.float32)
    for i in range(HD // 2):
        rotL[2 * i + 1, 2 * i] = -1.0
        rotL[2 * i, 2 * i + 1] = 1.0
    rotL = rotL.astype(bf16)

    r = np.arange(P)[:, None]
    c = np.arange(P)[None, :]
    trimask = (c >= r).astype(bf16)  # [128,128] upper-right valid

    wq_s = (wq * scale).astype(bf16)
    wk_s = wk.astype(bf16)
    wv_s = wv.astype(bf16)
    wo_s = wo.astype(bf16)

    in_maps = []
    for cix in range(NCORES):
        rows = slice(cix * CPC, (cix + 1) * CPC)  # head-channel rows
        blocks = []
        for h in range(HPC):
            hr = slice((cix * HPC + h) * HD, (cix * HPC + h + 1) * HD)
            blocks.append(wq_s[hr])   # q_h: [128, D]
        for h in range(HPC):
            hr = slice((cix * HPC + h) * HD, (cix * HPC + h + 1) * HD)
            blocks.append(wk_s[hr])
        blocks.append(wv_s[rows])     # v both heads: [256, D]
        wqkvT = np.concatenate(blocks, axis=0).T  # [D, 768] bf16
        # pack to SBUF image [p, dt, e]
        wqkvp = np.ascontiguousarray(
            wqkvT.reshape(NDT, P, 6 * P).transpose(1, 0, 2))
        woT = wo_s[:, rows].T  # [256, D] bf16
        wop = np.ascontiguousarray(
            woT.reshape(CPC // P, P, D).transpose(1, 0, 2))  # [p, o, e]
        in_maps.append({
            "xck": xck,
            "wqkvp": wqkvp,
            "wop": wop,
            "cosT": cosT,
            "sinT": sinT,
            "rotL": rotL,
            "trimask": trimask,
            "ones": np.ones((P, P), dtype=bf16),
        })
    return in_maps


def _get_nc():
    global _nc_cache
    if _nc_cache is None:
        _nc_cache = _build_nc()
    return _nc_cache


def kernel(x, wq, wk, wv, wo, _trace=False):
    global last_exec_time_ns
    nc = _get_nc()
    in_maps = _host_inputs(np.asarray(x, dtype=np.float32),
                           np.asarray(wq, dtype=np.float32),
                           np.asarray(wk, dtype=np.float32),
                           np.asarray(wv, dtype=np.float32),
                           np.asarray(wo, dtype=np.float32))
    res = run_bass_kernel_spmd(nc, in_maps, core_ids=list(range(NCORES)),
                               trace=_trace)
    last_exec_time_ns = res.exec_time_ns
    y = np.zeros((B, S, D), dtype=np.float64)
    for cix in range(NCORES):
        arr = res.results[cix]["yTp"].astype(np.float64)
        # [B, NSC, eg, P, sub, SC] -> [B, (NSC, SC), (eg, sub, P)]
        y += arr.transpose(0, 1, 5, 2, 4, 3).reshape(B, S, D)
    return y.astype(np.float32)


# revision 8
# speedup vs baseline: 1.3374x; 1.3374x over previous
"""Multi-head causal self-attention with RoPE on 8 Trainium2 NeuronCores.

Problem: x[2,2048,2048], wq/wk/wv/wo[2048,2048] fp32, 16 heads (hd=128),
interleaved RoPE, causal softmax, Megatron-style tensor parallelism over
heads: 2 heads per core, wo row-sharded, partial outputs summed on host.

All matmul operands are bf16 (measured end-to-end rel err ~3e-3 vs the
2e-2 gate); PSUM accumulation is fp32. bf16 stationaries get the
compiler's fast-weight-load path so LDWEIGHTS hides under the moving
stream, and all DMA volumes halve vs fp32.

Layout strategy (per core, per batch b):
  - host packs x chunk-major in exact SBUF image (xck[c] = [p, dt, s])
    so every chunk load is one DMA with 16KB-contiguous per-partition
    runs (max packet size); wqkv/wo likewise packed to SBUF image.
  - cold start: chunk-0 projections run dt-group-major across all 4
    qk psum chains + all 4 v chains simultaneously (8 PSUM banks), so
    the PE consumes x/w dt-groups as the interleaved startup DMAs land.
  - projections: qT,kT per head via lhsT=w-tile [d,e], rhs=x [d,s512]
    -> q^T,k^T [e=128, s] directly; v natural [s, e] via lhsT=x-subtile;
    RoPE fused right after each projection chunk (rot matmul + DVE,
    cos/sin tables in bf16 for 2x DVE mode).
  - attention per (b, j-block of 512 q), heads interleaved and scores
    pipelined three kv-tiles ahead so the exp (ACT) latency never stalls
    the PE:
      scoresT[kv=128, q<=512] = kT-tile.T @ qT-block
      attn = exp(scoresT) on ACT (bf16); triangle mask on 128-col band
      oT[d, q] += v-tile.T @ attn
      rowsum: full kv-tiles pair+quad-summed on DVE, one ones-matmul
      per quad; diagonal tiles matmul'd individually
      oT_norm = oT * reciprocal_approx_fast(rowsum) (DVE, bf16)
  - output projection per j-block: yT[e, jsl] = sum_ct woT-tile.T @ oT,
    copied PSUM->SBUF on DVE into [P,2,SC] tiles, one DMA per et-pair
  - host: y = sum over cores of yT^T
"""

import os
import sys

for _p in ("/opt/trn_rl_repo", "/root/.axon_site/_ro/trn_rl_repo"):
    if os.path.isdir(_p) and _p not in sys.path:
        sys.path.append(_p)

import numpy as np

import concourse.bacc as bacc
import concourse.mybir as mybir
import concourse.tile as tile
from concourse.alu_op_type import AluOpType
from concourse.bass_utils import run_bass_kernel_spmd

F32 = mybir.dt.float32
BF16 = mybir.dt.bfloat16

B, S, D = 2, 2048, 2048
H, HD = 16, 128
NCORES = 8
HPC = H // NCORES            # heads per core = 2
CPC = HPC * HD               # channels per core = 256
P = 128
SC = 512                     # s-chunk for projections / q-block for attention
NSC = S // SC                # 4
NDT = D // P                 # 16 contraction tiles
ROPE_THETA = 10000.0

# cold-start dt-groups: small first so the PE starts as soon as the first
# x/w tiles land, then steady 4-tile groups paced to the DMA delivery
XGRP = [(0, 2), (2, 2), (4, 4), (8, 4), (12, 4)]

Exp = mybir.ActivationFunctionType.Exp

last_exec_time_ns = None
_nc_cache = None


def _build_nc():
    nc = bacc.Bacc("TRN2", target_bir_lowering=False, debug=False)

    # host-packed SBUF-image inputs (contiguous per-partition runs)
    xck = nc.dram_tensor("xck", [B * NSC, P, NDT, SC], BF16, kind="ExternalInput")
    wqkvp = nc.dram_tensor("wqkvp", [P, NDT, 6 * P], BF16, kind="ExternalInput")
    wop = nc.dram_tensor("wop", [P, CPC // P, D], BF16, kind="ExternalInput")
    cosT = nc.dram_tensor("cosT", [HD, S], BF16, kind="ExternalInput")
    sinT = nc.dram_tensor("sinT", [HD, S], BF16, kind="ExternalInput")
    rotL = nc.dram_tensor("rotL", [HD, HD], BF16, kind="ExternalInput")
    trimask = nc.dram_tensor("trimask", [P, P], BF16, kind="ExternalInput")
    ones = nc.dram_tensor("ones", [P, P], BF16, kind="ExternalInput")
    yTp = nc.dram_tensor("yTp", [B, NSC, NDT // 4, P, 4, SC], BF16,
                         kind="ExternalOutput")

    with tile.TileContext(nc) as tc:
        with tc.tile_pool(name="const", bufs=1) as constp, \
             tc.tile_pool(name="xp", bufs=3) as xp, \
             tc.tile_pool(name="qk", bufs=2) as qkp, \
             tc.tile_pool(name="vp", bufs=2) as vp, \
             tc.tile_pool(name="op", bufs=2) as op_, \
             tc.tile_pool(name="attn", bufs=13) as attnp, \
             tc.tile_pool(name="asum", bufs=6) as sump, \
             tc.tile_pool(name="tmp", bufs=4) as tmpp, \
             tc.tile_pool(name="yt", bufs=5) as ytp, \
             tc.tile_pool(name="ps", bufs=4, space="PSUM") as psp, \
             tc.tile_pool(name="acc", bufs=4, space="PSUM") as accp:

            wq_sb = constp.tile([P, NDT, 6 * P], BF16)
            wo_sb = constp.tile([P, CPC // P, D], BF16)
            cos_sb = constp.tile([P, S], BF16)
            sin_sb = constp.tile([P, S], BF16)
            rot_sb = constp.tile([P, P], BF16)
            mask_sb = constp.tile([P, P], BF16)
            ones_sb = constp.tile([P, P], BF16)

            chunk_tiles = {}

            def chunk_dma(c, queue):
                xt = xp.tile([P, NDT, SC], BF16, tag="xt", name=f"xt{c}")
                queue.dma_start(xt[:], xck[c])
                chunk_tiles[c] = xt

            # ---- cold-start DMA schedule: interleave chunk-0 x and wqkv
            #      dt-groups round-robin over 4 queues so the group-major
            #      projection below is fed just-in-time; everything else
            #      (rot, chunk1, cos/sin, wo) queues behind in need order ----
            xt0 = xp.tile([P, NDT, SC], BF16, tag="xt", name="xt0")
            # w before x per group (PE starts with LDWEIGHTS); the scalar
            # queue joins late because its preamble (ACT_TABLE_LOAD) is
            # the longest
            GQ = [(nc.sync, nc.gpsimd), (nc.gpsimd, nc.sync),
                  (nc.scalar, nc.sync), (nc.gpsimd, nc.scalar),
                  (nc.sync, nc.gpsimd)]
            for (gs, gl), (wq_q, x_q) in zip(XGRP, GQ):
                wq_q.dma_start(wq_sb[:, gs:gs + gl, :], wqkvp[:, gs:gs + gl, :])
                x_q.dma_start(xt0[:, gs:gs + gl, :], xck[0, :, gs:gs + gl, :])
            chunk_tiles[0] = xt0
            qi = 0
            nc.sync.dma_start(rot_sb[:], rotL[:])
            nc.gpsimd.dma_start(mask_sb[:], trimask[:])
            nc.scalar.dma_start(ones_sb[:], ones[:])
            # chunk 1 split in 4 so its head tiles land before chunk-0
            # projections finish
            xt1 = xp.tile([P, NDT, SC], BF16, tag="xt", name="xt1")
            Q3 = [nc.sync, nc.gpsimd, nc.scalar]
            for g in range(4):
                Q3[g % 3].dma_start(xt1[:, 4 * g:4 * g + 4, :],
                                    xck[1, :, 4 * g:4 * g + 4, :])
            chunk_tiles[1] = xt1
            nc.scalar.dma_start(cos_sb[:], cosT[:])
            nc.sync.dma_start(sin_sb[:], sinT[:])
            nc.gpsimd.dma_start(wo_sb[:], wop[:])

            qkT = {}
            v_sb = {}

            def emit_rope(b, e, sl, pool):
                pr = pool.tile([P, SC], F32, tag="acc" if pool is accp else "ps")
                nc.tensor.matmul(pr[:], rot_sb[:], qkT[b, e][:, sl],
                                 start=True, stop=True)
                tmp = tmpp.tile([P, SC], BF16, tag="ropetmp")
                nc.vector.tensor_tensor(tmp[:], pr[:], sin_sb[:, sl],
                                        AluOpType.mult)
                nc.vector.tensor_tensor(qkT[b, e][:, sl], qkT[b, e][:, sl],
                                        cos_sb[:, sl], AluOpType.mult)
                nc.vector.tensor_tensor(qkT[b, e][:, sl], qkT[b, e][:, sl],
                                        tmp[:], AluOpType.add)

            def emit_proj_chunk0():
                """Cold-start chunk: dt-group-major across all 8 psum chains
                so the PE consumes x/w tiles in DMA arrival order."""
                xts = chunk_tiles.pop(0)
                sl = slice(0, SC)
                pqs = [psp.tile([P, SC], F32, tag="ps", name=f"pq0_{e}")
                       for e in range(4)]
                pvs = [accp.tile([P, SC], F32, tag="acc", name=f"pv0_{s}")
                       for s in range(4)]
                for (gs, gl) in XGRP:
                    for k in range(gs, gs + gl):
                        for e in range(4):
                            nc.tensor.matmul(pqs[e][:],
                                             wq_sb[:, k, e * P:(e + 1) * P],
                                             xts[:, k, :],
                                             start=(k == 0), stop=(k == NDT - 1),
                                             skip_group_check=True)
                        for ss in range(4):
                            nc.tensor.matmul(pvs[ss][:, :CPC],
                                             xts[:, k, ss * P:(ss + 1) * P],
                                             wq_sb[:, k, 4 * P:6 * P],
                                             start=(k == 0), stop=(k == NDT - 1),
                                             skip_group_check=True)
                for e in range(4):
                    nc.scalar.copy(qkT[0, e][:, sl], pqs[e][:])
                for ss in range(4):
                    nc.scalar.copy(v_sb[0][:, ss, :], pvs[ss][:, :CPC])
                for e in range(4):
                    emit_rope(0, e, sl, accp)
                chunk_dma(2, nc.sync)

            def emit_proj_chunk(b, sc, flush_pending):
                c = b * NSC + sc
                xts = chunk_tiles.pop(c)
                sl = slice(sc * SC, (sc + 1) * SC)
                for e in range(4):
                    pq = psp.tile([P, SC], F32, tag="ps")
                    for dt in range(NDT):
                        nc.tensor.matmul(pq[:],
                                         wq_sb[:, dt, e * P:(e + 1) * P],
                                         xts[:, dt, :],
                                         start=(dt == 0), stop=(dt == NDT - 1))
                    nc.scalar.copy(qkT[b, e][:, sl], pq[:])
                    if e == 0 and flush_pending is not None:
                        flush_pending()
                    if e > 0:
                        emit_rope(b, e - 1, sl, psp)
                for ss in range(SC // P):
                    pv = psp.tile([P, SC], F32, tag="ps")
                    pvv = pv[:, :CPC]
                    for dt in range(NDT):
                        nc.tensor.matmul(pvv,
                                         xts[:, dt, ss * P:(ss + 1) * P],
                                         wq_sb[:, dt, 4 * P:6 * P],
                                         start=(dt == 0), stop=(dt == NDT - 1))
                    if ss == 0:
                        emit_rope(b, 3, sl, psp)
                    nc.scalar.copy(v_sb[b][:, sc * (SC // P) + ss, :], pvv)
                # prefetch chunk c+2 only after chunk c's readers are emitted,
                # so the buffer-reuse WAR dependency is tracked
                if c + 2 < B * NSC:
                    chunk_dma(c + 2, nc.gpsimd if c % 2 else nc.sync)

            pending_oproj = []

            def flush_oproj(n=None):
                k = len(pending_oproj) if n is None else min(n, len(pending_oproj))
                for _ in range(k):
                    pending_oproj.pop(0)()

            oT = {}
            for b in range(B):
                for e in range(4):
                    qkT[b, e] = qkp.tile([P, S], BF16, tag=f"qk{e}",
                                         name=f"qkT{b}_{e}")
                v_sb[b] = vp.tile([P, NDT, CPC], BF16, tag="v", name=f"v{b}")
                for sc in range(NSC):
                    if b == 0 and sc == 0:
                        emit_proj_chunk0()
                    else:
                        emit_proj_chunk(b, sc,
                                        flush_oproj if sc == 0 else None)

                # ---- attention: j outer, heads interleaved, scores three
                #      kv-tiles ahead so exp latency is hidden; each j's
                #      output projection is deferred into the next j-block's
                #      (or batch's) pipeline so the PE never waits on the
                #      DVE softmax-normalization chain ----
                oT[b] = [op_.tile([P, S], BF16, tag=f"o{h}", name=f"oT{b}_{h}")
                         for h in range(HPC)]

                def emit_score(j, t, h, b=b):
                    dp = t - (SC // P) * j
                    dlt = max(dp, 0) * P
                    qsl = slice(j * SC + dlt, (j + 1) * SC)
                    pscore = psp.tile([P, SC], F32, tag="ps")
                    nc.tensor.matmul(pscore[:, dlt:],
                                     qkT[b, 2 + h][:, t * P:(t + 1) * P],
                                     qkT[b, h][:, qsl],
                                     start=True, stop=True)
                    at = attnp.tile([P, SC], BF16, tag="attn")
                    nc.scalar.activation(at[:, dlt:], pscore[:, dlt:],
                                         Exp, bias=0.0, scale=1.0)
                    if dp >= 0:  # triangle mask on the 128-col band
                        # gpsimd: its queue is idle in attention windows, so
                        # the masked tile is ready for AV without waiting
                        # behind DVE's cast backlog
                        nc.gpsimd.tensor_tensor(
                            at[:, dlt:dlt + P], at[:, dlt:dlt + P],
                            mask_sb[:], AluOpType.mult)
                    return at

                def make_oproj(j, b=b, in_proj=False, final=False):
                    jsl = slice(j * SC, (j + 1) * SC)

                    store_q = ([nc.sync, nc.gpsimd, nc.scalar]
                               if final else [nc.sync, nc.gpsimd])

                    def emit_eg(eg):
                        # PSUM->SBUF casts 3:1 DVE:ACT (ACT only has ~8us
                        # of exp slack per window); one contiguous
                        # [P,4,SC] store DMA per 4 ets
                        yt = ytp.tile([P, 4, SC], BF16, tag="yt")
                        for sub in range(4):
                            et = 4 * eg + sub
                            py = psp.tile([P, SC], F32, tag="ps")
                            for ct in range(HPC):
                                nc.tensor.matmul(
                                    py[:],
                                    wo_sb[:, ct, et * P:(et + 1) * P],
                                    oT[b][ct][:, jsl],
                                    start=(ct == 0), stop=(ct == HPC - 1))
                            if et % 4 == 3:
                                nc.scalar.copy(yt[:, sub, :], py[:])
                            else:
                                nc.vector.tensor_copy(yt[:, sub, :], py[:])
                        store_q[eg % len(store_q)].dma_start(
                            yTp[b, j, eg], yt[:])
                    return [lambda eg=eg: emit_eg(eg) for eg in range(NDT // 4)]

                LA = 4  # score lookahead in kv-tiles
                att_q = {}  # (t, h) -> attn tile, pipelined
                at_prev = {}  # stashed full attn tiles awaiting pair-sum
                pair_prev = {}  # stashed pair-sums awaiting quad-sum
                for tp in range(LA):
                    for h in range(HPC):
                        att_q[0, tp, h] = emit_score(0, tp, h)

                for j in range(NSC):
                    jsl = slice(j * SC, (j + 1) * SC)
                    n_kv = (SC // P) * (j + 1)
                    po = [accp.tile([P, SC], F32, tag="acc", name=f"po{h}")
                          for h in range(HPC)]
                    prs = [accp.tile([P, SC], F32, tag="acc", name=f"prs{h}")
                           for h in range(HPC)]
                    for t in range(n_kv):
                        nxt = t + LA
                        if nxt < n_kv:
                            for h in range(HPC):
                                att_q[j, nxt, h] = emit_score(j, nxt, h)
                        elif j + 1 < NSC:
                            for h in range(HPC):
                                att_q[j + 1, nxt - n_kv, h] = emit_score(
                                    j + 1, nxt - n_kv, h)
                        dlt = max(t - (SC // P) * j, 0) * P
                        full = t < (SC // P) * j
                        for h in range(HPC):
                            at = att_q.pop((j, t, h))
                            nc.tensor.matmul(po[h][:, dlt:],
                                             v_sb[b][:, t, h * HD:(h + 1) * HD],
                                             at[:, dlt:],
                                             start=(t == 0), stop=(t == n_kv - 1),
                                             skip_group_check=True)
                            if full and t % 2 == 0:
                                at_prev[h] = at
                            elif full and t % 4 == 1:
                                # first pair of a quad: sum on DVE, stash
                                s2 = sump.tile([P, SC], BF16, tag="asum",
                                               name=f"asum{h}")
                                nc.vector.tensor_tensor(
                                    s2[:], at_prev.pop(h)[:], at[:],
                                    AluOpType.add)
                                pair_prev[h] = s2
                            elif full:
                                # second pair: fold into the stashed pair-sum,
                                # then one rowsum matmul streams 1/4 of the
                                # full-tile elements
                                s2 = sump.tile([P, SC], BF16, tag="asum",
                                               name=f"asum{h}")
                                nc.vector.tensor_tensor(
                                    s2[:], at_prev.pop(h)[:], at[:],
                                    AluOpType.add)
                                q4 = pair_prev.pop(h)
                                nc.vector.tensor_tensor(
                                    q4[:], q4[:], s2[:], AluOpType.add)
                                nc.tensor.matmul(prs[h][:], ones_sb[:], q4[:],
                                                 start=(t == 3), stop=False,
                                                 skip_group_check=True)
                            else:
                                nc.tensor.matmul(prs[h][:, dlt:], ones_sb[:],
                                                 at[:, dlt:],
                                                 start=(t == 0),
                                                 stop=(t == n_kv - 1),
                                                 skip_group_check=True)
                        if t >= 1:
                            # spread the deferred output projection across
                            # t-steps so the DVE/ACT cast load trickles in
                            # behind the exp/pair-add traffic
                            flush_oproj(None if t == n_kv - 1 else 1)
                    for h in range(HPC):
                        recip = tmpp.tile([P, SC], F32, tag="recip")
                        nc.vector.reciprocal_approx_fast(recip[:], prs[h][:])
                        nc.vector.tensor_tensor(oT[b][h][:, jsl], po[h][:],
                                                recip[:], AluOpType.mult)
                    pending_oproj.extend(make_oproj(
                        j, in_proj=(j == NSC - 1),
                        final=(b == B - 1 and j == NSC - 1)))
            flush_oproj()
    nc.finalize()
    return nc


def _host_inputs(x, wq, wk, wv, wo):
    """Build per-core input maps (host-side shard + SBUF-image packing)."""
    import ml_dtypes
    bf16 = ml_dtypes.bfloat16
    scale = 1.0 / np.sqrt(np.float32(HD))

    # x packed chunk-major in SBUF image: xck[b*NSC+sc] = [p, dt, s]
    xr = x.reshape(B, NSC, SC, NDT, P)            # (b, sc, s, o, p)
    xck = np.ascontiguousarray(
        xr.transpose(0, 1, 4, 3, 2).reshape(B * NSC, P, NDT, SC)).astype(bf16)

    # RoPE tables in [e, s] layout (same for every head), bf16 for 2x DVE
    inv_freq = 1.0 / (ROPE_THETA ** (np.arange(0, HD, 2, dtype=np.float64) / HD))
    ang = np.arange(S, dtype=np.float64)[None, :] * inv_freq[:, None]  # [64, S]
    cosT = np.repeat(np.cos(ang), 2, axis=0).astype(bf16)  # [128, S]
    sinT = np.repeat(np.sin(ang), 2, axis=0).astype(bf16)

    # signed pair-swap: qrot[2i] = -q[2i+1], qrot[2i+1] = q[2i]
    # matmul computes qrot[m, s] = sum_k rotL[k, m] q[k, s]
    rotL = np.zeros((HD, HD), dtype=np.float32)
    for i in range(HD // 2):
        rotL[2 * i + 1, 2 * i] = -1.0
        rotL[2 * i, 2 * i + 1] = 1.0
    rotL = rotL.astype(bf16)

    r = np.arange(P)[:, None]
    c = np.arange(P)[None, :]
    trimask = (c >= r).astype(bf16)  # [128,128] upper-right valid

    wq_s = (wq * scale).astype(bf16)
    wk_s = wk.astype(bf16)
    wv_s = wv.astype(bf16)
    wo_s = wo.astype(bf16)

    in_maps = []
    for cix in range(NCORES):
        rows = slice(cix * CPC, (cix + 1) * CPC)  # head-channel rows
        blocks = []
        for h in range(HPC):
            hr = slice((cix * HPC + h) * HD, (cix * HPC + h + 1) * HD)
            blocks.append(wq_s[hr])   # q_h: [128, D]
        for h in range(HPC):
            hr = slice((cix * HPC + h) * HD, (cix * HPC + h + 1) * HD)
            blocks.append(wk_s[hr])
        blocks.append(wv_s[rows])     # v both heads: [256, D]
        wqkvT = np.concatenate(blocks, axis=0).T  # [D, 768] bf16
        # pack to SBUF image [p, dt, e]
        wqkvp = np.ascontiguousarray(
            wqkvT.reshape(NDT, P, 6 * P).transpose(1, 0, 2))
        woT = wo_s[:, rows].T  # [256, D] bf16
        wop = np.ascontiguousarray(
            woT.reshape(CPC // P, P, D).transpose(1, 0, 2))  # [p, o, e]
        in_maps.append({
            "xck": xck,
            "wqkvp": wqkvp,
            "wop": wop,
            "cosT": cosT,
            "sinT": sinT,
            "rotL": rotL,
            "trimask": trimask,
            "ones": np.ones((P, P), dtype=bf16),
        })
    return in_maps


def _get_nc():
    global _nc_cache
    if _nc_cache is None:
        _nc_cache = _build_nc()
    return _nc_cache


def kernel(x, wq, wk, wv, wo, _trace=False):
    global last_exec_time_ns
    nc = _get_nc()
    in_maps = _host_inputs(np.asarray(x, dtype=np.float32),
                           np.asarray(wq, dtype=np.float32),
                           np.asarray(wk, dtype=np.float32),
                           np.asarray(wv, dtype=np.float32),
                           np.asarray(wo, dtype=np.float32))
    res = run_bass_kernel_spmd(nc, in_maps, core_ids=list(range(NCORES)),
                               trace=_trace)
    last_exec_time_ns = res.exec_time_ns
    y = np.zeros((B, S, D), dtype=np.float64)
    for cix in range(NCORES):
        arr = res.results[cix]["yTp"].astype(np.float64)
        # [B, NSC, eg, P, sub, SC] -> [B, (NSC, SC), (eg, sub, P)]
        y += arr.transpose(0, 1, 5, 2, 4, 3).reshape(B, S, D)
    return y.astype(np.float32)
